# revision 1
# baseline (speedup 1.0000x reference)
"""Fused varlen SigLIP attention block for TRN2, tensor-parallel over heads
across 8 NeuronCores (2 heads per core).

Per core (heads 2c, 2c+1):
  - host pre-transposes x -> xT [H, T] (bf16); all matmuls stream xT.
  - qkvT per t-tile: psum[tl, 432] = xT_tile.T @ wqkvT  (cols q0 q1 k0 k1 v0 v1)
  - rope in t-major on strided [4, 36] half views; PE-transpose q,k per
    72-col tensor into QK [72, 4, T] bf16 (d-major).
  - v evacuated s-major into vseg [tl, 146] = [v0|1|v1|1]; the ones column
    makes the PV matmul emit the softmax row-sum as ctx row 72.
  - per segment/chunk(<=512)/s-tile(<=128): scoresT=kT.T@qT, exp on ACT
    (scale=1/sqrt(72), bias=-4 cancels in normalization), PV accumulates
    ctxT[73, tn]; normalize by DMA-broadcast 1/rowsum.
  - out-proj row-parallel: outT_partial[1152, T]; host sums the 8 partials.

Tiling is segment-aligned from cu_seqlens values (any sorted values work);
the BIR is specialized per plan and cached. bqkv/bout are zeros per spec;
bout is still added on the host.
"""
import numpy as np
from contextlib import ExitStack

import ml_dtypes
import concourse.bass as bass
import concourse.bacc as bacc
import concourse.tile as tile
import concourse.mybir as mybir
from concourse import bass_utils

F32 = mybir.dt.float32
BF16 = mybir.dt.bfloat16

H = 1152
NH = 16
HD = 72
HD2 = 36
T = 4096
NCORES = 8
HPC = NH // NCORES          # heads per core
OC = 3 * HPC * HD           # 432
SCALE = HD ** -0.5
EXP_BIAS = -4.0

_CACHE = {}


def _plan(cu):
    bs = sorted(set([0, T] + [int(v) for v in cu[1:] if 0 < int(v) < T]))
    segs = [(a, b) for a, b in zip(bs[:-1], bs[1:]) if b > a]
    plan = []
    for (a, b) in segs:
        chunks = []
        c0 = a
        while c0 < b:
            cn = min(512, b - c0)
            tls = []
            t0 = c0
            while t0 < c0 + cn:
                tl = min(128, c0 + cn - t0)
                tls.append((t0, tl))
                t0 += tl
            chunks.append((c0, cn, tuple(tls)))
            c0 += cn
        plan.append((a, b, tuple(chunks)))
    return tuple(plan)


def _all_tiles(plan):
    out = []
    for a, b, chunks in plan:
        for c0, cn, tls in chunks:
            out.extend(tls)
    return out


def build(nc, plan):
    tiles = _all_tiles(plan)
    nt = len(tiles)
    tidx = {t0: i for i, (t0, tl) in enumerate(tiles)}

    x_t = nc.dram_tensor("x_t", [H, T], BF16, kind="ExternalInput").ap()
    wq_t = nc.dram_tensor("wq_t", [H, OC], BF16, kind="ExternalInput").ap()
    wo_t = nc.dram_tensor("wo_t", [HPC, HD, H], BF16, kind="ExternalInput").ap()
    cs4d = nc.dram_tensor("cs4d", [nt, 128, 288], F32, kind="ExternalInput").ap()
    idd = nc.dram_tensor("idd", [128, 128], BF16, kind="ExternalInput").ap()
    outT = nc.dram_tensor("outT", [H, T], F32, kind="ExternalOutput").ap()

    with tile.TileContext(nc) as tc, ExitStack() as ctx:
        P = lambda **kw: ctx.enter_context(tc.tile_pool(**kw))
        singles = P(name="singles", bufs=1)
        xin = P(name="xin", bufs=2)
        stp = P(name="stp", bufs=3)
        tmp = P(name="tmp", bufs=2)
        esp = P(name="esp", bufs=3)
        cxp = P(name="cxp", bufs=5)
        bcp = P(name="bcp", bufs=2)
        osb = P(name="osb", bufs=4)
        ps_qkv = P(name="ps_qkv", bufs=2, space="PSUM")
        ps_tp = P(name="ps_tp", bufs=1, space="PSUM")
        ps_sc = P(name="ps_sc", bufs=3, space="PSUM")
        ps_cx = P(name="ps_cx", bufs=1, space="PSUM")
        ps_ou = P(name="ps_ou", bufs=1, space="PSUM")

        wq_sb = singles.tile([128, 9, OC], BF16)
        nc.sync.dma_start(out=wq_sb, in_=wq_t.rearrange("(kt p) m -> p kt m", p=128))
        wo_sb = singles.tile([HD, HPC, H], BF16)
        nc.sync.dma_start(out=wo_sb, in_=wo_t.rearrange("h d o -> d h o"))
        cs4 = singles.tile([128, nt, 288], F32)
        ident = singles.tile([128, 128], BF16)
        nc.sync.dma_start(out=ident, in_=idd)
        ebias = singles.tile([128, 1], F32)
        nc.vector.memset(ebias, EXP_BIAS)
        QK = singles.tile([HD, 4, T], BF16)
        vseg = singles.tile([128, nt, 194], BF16)   # per head: v(72) z(24) one(1)

        xts = {}

        def load_chunk(c0, cn, tls):
            xt = xin.tile([128, 9, 512], BF16, tag="xt", name=f"xt_{c0}")
            nc.sync.dma_start(
                out=xt[:, :, :cn],
                in_=x_t.rearrange("(kt p) t -> p kt t", p=128)[:, :, c0:c0 + cn])
            for (t0, tl) in tls:
                xts[t0] = (xt, t0 - c0)
                i = tidx[t0]
                nc.sync.dma_start(out=cs4[:, i, :], in_=cs4d[i])

        def qkv_mm(t0, tl):
            i = tidx[t0]
            ps = ps_qkv.tile([128, OC], F32, tag="psq", name=f"psq_{i}")
            xt, off = xts[t0]
            for kt in range(9):
                nc.tensor.matmul(ps[:tl, :], xt[:, kt, off:off + tl],
                                 wq_sb[:, kt, :], start=(kt == 0), stop=(kt == 8))
            return ps

        def rope_tp(t0, tl, ps):
            i = tidx[t0]
            qk = ps[:tl, 0:288].rearrange("p (j h d) -> p j h d", h=2, d=36)
            px1 = qk[:, :, 0, :]
            px2 = qk[:, :, 1, :]
            c = cs4[:tl, i, 0:144].rearrange("p (j d) -> p j d", d=36)
            s = cs4[:tl, i, 144:288].rearrange("p (j d) -> p j d", d=36)
            m1 = tmp.tile([128, 4, 36], F32, tag="m1", name=f"m1_{i}")
            m2 = tmp.tile([128, 4, 36], F32, tag="m2", name=f"m2_{i}")
            m3 = tmp.tile([128, 4, 36], F32, tag="m3", name=f"m3_{i}")
            m4 = tmp.tile([128, 4, 36], F32, tag="m4", name=f"m4_{i}")
            nc.vector.tensor_tensor(out=m1[:tl], in0=px1, in1=c, op=mybir.AluOpType.mult)
            nc.vector.tensor_tensor(out=m2[:tl], in0=px2, in1=s, op=mybir.AluOpType.mult)
            nc.vector.tensor_tensor(out=m3[:tl], in0=px2, in1=c, op=mybir.AluOpType.mult)
            nc.vector.tensor_tensor(out=m4[:tl], in0=px1, in1=s, op=mybir.AluOpType.mult)
            stg = stp.tile([128, 4, 2, 36], BF16, tag="stg", name=f"stg_{i}")
            nc.gpsimd.tensor_tensor(out=stg[:tl, :, 0, :], in0=m1[:tl], in1=m2[:tl],
                                    op=mybir.AluOpType.subtract)
            nc.gpsimd.tensor_tensor(out=stg[:tl, :, 1, :], in0=m3[:tl], in1=m4[:tl],
                                    op=mybir.AluOpType.add)
            pt = ps_tp.tile([HD, 512], BF16, tag="pt", name=f"pt_{i}")
            stgf = stg.rearrange("p j h d -> p (j h d)")
            for j in range(4):
                nc.tensor.transpose(pt[:, j * tl:(j + 1) * tl],
                                    stgf[:tl, j * 72:(j + 1) * 72], ident[:tl, :tl])
            nc.vector.tensor_copy(QK[:, :, t0:t0 + tl],
                                  pt[:, 0:4 * tl].rearrange("d (j t) -> d j t", j=4))
            nc.scalar.copy(vseg[:tl, i, 0:72], ps[:tl, 288:360])
            nc.scalar.copy(vseg[:tl, i, 97:169], ps[:tl, 360:432])
            nc.gpsimd.memset(vseg[:tl, i, 72:96], 0.0)
            nc.gpsimd.memset(vseg[:tl, i, 96:97], 1.0)
            nc.gpsimd.memset(vseg[:tl, i, 169:193], 0.0)
            nc.gpsimd.memset(vseg[:tl, i, 193:194], 1.0)

        pending = None
        for a, b, chunks in plan:
            for c0, cn, tls in chunks:
                load_chunk(c0, cn, tls)
                for (t0, tl) in tls:
                    ps = qkv_mm(t0, tl)
                    if pending is not None:
                        rope_tp(*pending)
                    pending = (t0, tl, ps)
        if pending is not None:
            rope_tp(*pending)

        # ---------------- phase 2: attention + out-proj ------------------
        def pv_do(cx, cn, st, h, first, last):
            s0, sn, es = st
            i = tidx[s0]
            nc.tensor.matmul(cx[:, :cn], vseg[:sn, i, h * 97:(h + 1) * 97],
                             es[:sn, :cn], start=first, stop=last)

        def attn_chunk(a, b, c0, cn):
            sts = []
            s0 = a
            while s0 < b:
                sn = min(128, b - s0)
                sts.append((s0, sn))
                s0 += sn
            ctxs = []
            for h in range(HPC):
                cx = ps_cx.tile([97, 512], F32, tag="cx", name=f"cx_{c0}_{h}")
                prev = None
                for si, (s0, sn) in enumerate(sts):
                    sc = ps_sc.tile([128, 512], F32, tag="sc", name=f"sc_{c0}_{h}_{si}")
                    nc.tensor.matmul(sc[:sn, :cn], QK[:, 2 + h, s0:s0 + sn],
                                     QK[:, h, c0:c0 + cn], start=True, stop=True)
                    es = esp.tile([128, 512], BF16, tag="es", name=f"es_{c0}_{h}_{si}")
                    nc.scalar.activation(es[:sn, :cn], sc[:sn, :cn],
                                         mybir.ActivationFunctionType.Exp,
                                         bias=ebias[:sn], scale=SCALE)
                    if prev is not None:
                        pv_do(cx, cn, prev, h, first=(si == 1), last=False)
                    prev = (s0, sn, es)
                pv_do(cx, cn, prev, h, first=(len(sts) == 1), last=True)
                rs = bcp.tile([1, 512], F32, tag="rs", name=f"rs_{c0}_{h}")
                nc.scalar.copy(rs[:, :cn], cx[96:97, :cn])
                rr = bcp.tile([1, 512], F32, tag="rr", name=f"rr_{c0}_{h}")
                nc.vector.reciprocal_approx_fast(out=rr[:, :cn], in_=rs[:, :cn])
                bc = bcp.tile([HD, 512], F32, tag="bc", name=f"bc_{c0}_{h}")
                nc.gpsimd.partition_broadcast(bc[:, :cn], rr[:, :cn])
                cxs = cxp.tile([HD, 512], BF16, tag="cxs", name=f"cxs_{c0}_{h}")
                nc.vector.tensor_tensor(out=cxs[:, :cn], in0=cx[0:HD, :cn],
                                        in1=bc[:, :cn], op=mybir.AluOpType.mult)
                ctxs.append(cxs)
            return ctxs

        def outproj(c0, cn, ctxs):
            for m in range(9):
                po = ps_ou.tile([128, 512], F32, tag="po", name=f"po_{c0}_{m}")
                for h in range(HPC):
                    nc.tensor.matmul(po[:, :cn], wo_sb[:, h, m * 128:(m + 1) * 128],
                                     ctxs[h][:, :cn], start=(h == 0), stop=(h == HPC - 1))
                ob = osb.tile([128, 512], F32, tag="ob", name=f"ob_{c0}_{m}")
                if m % 2 == 0:
                    nc.scalar.copy(ob[:, :cn], po[:, :cn])
                else:
                    nc.vector.tensor_copy(ob[:, :cn], po[:, :cn])
                nc.sync.dma_start(out=outT[m * 128:(m + 1) * 128, c0:c0 + cn],
                                  in_=ob[:, :cn])

        pend_out = None
        for a, b, chunks in plan:
            for c0, cn, tls in chunks:
                ctxs = attn_chunk(a, b, c0, cn)
                if pend_out is not None:
                    outproj(*pend_out)
                pend_out = (c0, cn, ctxs)
        if pend_out is not None:
            outproj(*pend_out)
    return nc


def _build_inputs(x, wqkv, wout, cos, sin, plan):
    tiles = _all_tiles(plan)
    nt = len(tiles)
    bf = ml_dtypes.bfloat16
    x_t = np.ascontiguousarray(x.T).astype(bf)
    c = cos[:, :HD2]
    s = sin[:, :HD2]
    cs4d = np.zeros((nt, 128, 288), np.float32)
    for i, (t0, tl) in enumerate(tiles):
        cs4d[i, :tl, 0:144] = np.tile(c[t0:t0 + tl], (1, 4))
        cs4d[i, :tl, 144:288] = np.tile(s[t0:t0 + tl], (1, 4))
    idd = np.eye(128, dtype=np.float32).astype(bf)

    in_maps = []
    for core in range(NCORES):
        h0 = core * HPC
        rows = []
        for kind in range(3):
            for h in range(HPC):
                base = kind * H + (h0 + h) * HD
                rows.extend(range(base, base + HD))
        wq = np.ascontiguousarray(wqkv[rows, :].T).astype(bf)      # [H, 432]
        cols = np.arange(h0 * HD, (h0 + HPC) * HD)
        wo = np.ascontiguousarray(wout[:, cols].T).astype(bf)      # [144, H]
        wo = np.ascontiguousarray(wo.reshape(HPC, HD, H))
        in_maps.append({"x_t": x_t, "wq_t": wq, "wo_t": wo,
                        "cs4d": cs4d, "idd": idd})
    return in_maps


def kernel(hidden_states, wqkv, bqkv, wout, bout, cos, sin, cu_seqlens,
           _trace=False):
    x = np.asarray(hidden_states, np.float32).reshape(T, H)
    plan = _plan(np.asarray(cu_seqlens).astype(np.int64))
    if plan not in _CACHE:
        nc = bacc.Bacc("TRN2", target_bir_lowering=False, debug=False)
        build(nc, plan)
        nc.compile()
        _CACHE[plan] = nc
    nc = _CACHE[plan]
    in_maps = _build_inputs(x, np.asarray(wqkv, np.float32),
                            np.asarray(wout, np.float32),
                            np.asarray(cos, np.float32),
                            np.asarray(sin, np.float32), plan)
    res = bass_utils.run_bass_kernel_spmd(nc, in_maps,
                                          core_ids=list(range(NCORES)),
                                          trace=_trace)
    out = np.zeros((H, T), np.float64)
    for core in range(NCORES):
        out += res.results[core]["outT"].astype(np.float64)
    out = out.T + np.asarray(bout, np.float64)[None, :]
    if _trace:
        kernel.last_exec_time_ns = res.exec_time_ns
        kernel.last_trace = res.instructions_and_trace
    return out.astype(np.float32).reshape(1, T, H)



# revision 6
# speedup vs baseline: 1.4275x; 1.4275x over previous
"""Fused varlen SigLIP attention block for TRN2, tensor-parallel over heads
across 8 NeuronCores (2 heads per core).

v2: optimized for dense PE streams.
  - All inputs preloaded up-front in host-packed per-partition-contiguous
    layouts (few large DMA descriptors); no per-tile DMA in the main loops.
  - Phase 1 per t-tile: qkv psum[tl,432] (9 matmuls), rope split DVE/GpSimd
    with 0-stride broadcast cos/sin views, 4 PE transposes -> QK[72,4,T],
    v evacuated by ACT (idle in phase 1) into vseg with a ones column.
  - Phase 2 per segment: chunk PAIRS share one [128,1024] 2-bank score psum
    so exp runs as one wide ACT instr; PV deferred one s-tile behind scores;
    normalize split into early bank-freeing copies + recip/broadcast/mult
    off the critical path; outproj deferred one pair; evac on DVE.
  - PSUM: phase1 qkv(3)+tp(2) banks; phase2 sc(2x2)+cx(2)+out(2) banks.

bqkv/bout are zeros per spec; bout is still added on the host.
"""
import numpy as np
from contextlib import ExitStack

import ml_dtypes
import concourse.bass as bass
import concourse.bacc as bacc
import concourse.tile as tile
import concourse.mybir as mybir
from concourse import bass_utils

F32 = mybir.dt.float32
BF16 = mybir.dt.bfloat16

H = 1152
NH = 16
HD = 72
HD2 = 36
T = 4096
NCORES = 8
HPC = NH // NCORES          # heads per core
OC = 3 * HPC * HD           # 432
SCALE = HD ** -0.5
EXP_BIAS = -4.0

_CACHE = {}


def _plan(cu):
    bs = sorted(set([0, T] + [int(v) for v in cu[1:] if 0 < int(v) < T]))
    segs = [(a, b) for a, b in zip(bs[:-1], bs[1:]) if b > a]
    plan = []
    for (a, b) in segs:
        chunks = []
        c0 = a
        while c0 < b:
            cn = min(512, b - c0)
            tls = []
            t0 = c0
            while t0 < c0 + cn:
                tl = min(128, c0 + cn - t0)
                tls.append((t0, tl))
                t0 += tl
            chunks.append((c0, cn, tuple(tls)))
            c0 += cn
        plan.append((a, b, tuple(chunks)))
    return tuple(plan)


def _all_tiles(plan):
    out = []
    for a, b, chunks in plan:
        for c0, cn, tls in chunks:
            out.extend(tls)
    return out


def _stiles(a, b):
    sts = []
    s0 = a
    while s0 < b:
        sn = min(128, b - s0)
        sts.append((s0, sn))
        s0 += sn
    return sts


def _bcast(ap2d, n):
    """[p, d] view -> [p, n, d] with 0-stride middle dim."""
    return bass.AP(ap2d.tensor, ap2d.offset, [ap2d.ap[0], [0, n], ap2d.ap[1]])


def build(nc, plan):
    tiles = _all_tiles(plan)
    nt = len(tiles)
    tidx = {t0: i for i, (t0, tl) in enumerate(tiles)}

    xP = nc.dram_tensor("xP", [128, 9, T], BF16, kind="ExternalInput").ap()
    wqP = nc.dram_tensor("wqP", [128, 9, OC], BF16, kind="ExternalInput").ap()
    woP = nc.dram_tensor("woP", [HD, HPC, H], BF16, kind="ExternalInput").ap()
    csP = nc.dram_tensor("csP", [128, nt, 2 * HD2], BF16, kind="ExternalInput").ap()
    idd = nc.dram_tensor("idd", [128, 128], BF16, kind="ExternalInput").ap()
    outT = nc.dram_tensor("outT", [H, T], F32, kind="ExternalOutput").ap()

    with tile.TileContext(nc) as tc, ExitStack() as ctx:
        P = lambda **kw: ctx.enter_context(tc.tile_pool(**kw))
        singles = P(name="singles", bufs=1)

        xsb = singles.tile([128, 9, T], BF16)
        wq_sb = singles.tile([128, 9, OC], BF16)
        wo_sb = singles.tile([HD, HPC, H], BF16)
        cssb = singles.tile([128, nt, 2 * HD2], BF16)
        ident = singles.tile([128, 128], BF16)
        ebias = singles.tile([128, 1], F32)
        QK = singles.tile([HD, 4, T], BF16)
        vseg = singles.tile([128, nt, 194], BF16)   # per head: v(72) z(24) one(1)

        for kt in range(9):
            nc.sync.dma_start(out=xsb[:, kt, :], in_=xP[:, kt, :])
        nc.sync.dma_start(out=wq_sb, in_=wqP)
        nc.sync.dma_start(out=wo_sb, in_=woP)
        nc.sync.dma_start(out=cssb, in_=csP)
        nc.sync.dma_start(out=ident, in_=idd)
        nc.vector.memset(ebias, EXP_BIAS)
        # ones/zeros zones of vseg, set once for all tiles
        nc.gpsimd.memset(vseg[:, :, 72:96], 0.0)
        nc.gpsimd.memset(vseg[:, :, 96:97], 1.0)
        nc.gpsimd.memset(vseg[:, :, 169:193], 0.0)
        nc.gpsimd.memset(vseg[:, :, 193:194], 1.0)

        # ---------------- phase 1: qkv + rope + transpose ----------------
        with ExitStack() as p1:
            P1 = lambda **kw: p1.enter_context(tc.tile_pool(**kw))
            ps_qkv = P1(name="ps_qkv", bufs=3, space="PSUM")
            ps_tp = P1(name="ps_tp", bufs=2, space="PSUM")
            tmpd = P1(name="tmpd", bufs=3)
            tmpg = P1(name="tmpg", bufs=3)
            stp = P1(name="stp", bufs=3)

            def qkv_mm(t0, tl):
                i = tidx[t0]
                ps = ps_qkv.tile([128, OC], F32, tag="psq", name=f"psq_{i}")
                for kt in range(9):
                    nc.tensor.matmul(ps[:tl, :], xsb[:, kt, t0:t0 + tl],
                                     wq_sb[:, kt, :], start=(kt == 0),
                                     stop=(kt == 8))
                return ps

            def rope_tp(t0, tl, ps):
                i = tidx[t0]
                qk = ps[:tl, 0:288].rearrange("p (j h d) -> p j h d", h=2, d=36)
                px1 = qk[:, :, 0, :]
                px2 = qk[:, :, 1, :]
                c = _bcast(cssb[:tl, i, 0:36], 4)
                s = _bcast(cssb[:tl, i, 36:72], 4)
                m1 = tmpd.tile([128, 4, 36], F32, tag="m1", name=f"m1_{i}")
                m2 = tmpd.tile([128, 4, 36], F32, tag="m2", name=f"m2_{i}")
                m3 = tmpg.tile([128, 4, 36], F32, tag="m3", name=f"m3_{i}")
                m4 = tmpg.tile([128, 4, 36], F32, tag="m4", name=f"m4_{i}")
                stg = stp.tile([128, 4, 2, 36], BF16, tag="stg", name=f"stg_{i}")
                # GpSimd cannot read PSUM: DVE does the 4 psum-side mults,
                # GpSimd combines the SBUF intermediates.
                nc.vector.tensor_tensor(out=m1[:tl], in0=px1, in1=c,
                                        op=mybir.AluOpType.mult)
                nc.vector.tensor_tensor(out=m2[:tl], in0=px2, in1=s,
                                        op=mybir.AluOpType.mult)
                nc.vector.tensor_tensor(out=m3[:tl], in0=px2, in1=c,
                                        op=mybir.AluOpType.mult)
                nc.vector.tensor_tensor(out=m4[:tl], in0=px1, in1=s,
                                        op=mybir.AluOpType.mult)
                nc.gpsimd.tensor_tensor(out=stg[:tl, :, 0, :], in0=m1[:tl],
                                        in1=m2[:tl], op=mybir.AluOpType.subtract)
                nc.gpsimd.tensor_tensor(out=stg[:tl, :, 1, :], in0=m3[:tl],
                                        in1=m4[:tl], op=mybir.AluOpType.add)
                pt = ps_tp.tile([HD, 512], BF16, tag="pt", name=f"pt_{i}")
                stgf = stg.rearrange("p j h d -> p (j h d)")
                for j in range(4):
                    nc.tensor.transpose(pt[:, j * tl:(j + 1) * tl],
                                        stgf[:tl, j * 72:(j + 1) * 72],
                                        ident[:tl, :tl])
                nc.vector.tensor_copy(QK[:, :, t0:t0 + tl],
                                      pt[:, 0:4 * tl].rearrange(
                                          "d (j t) -> d j t", j=4))
                # v evacuation on ACT (idle during phase 1)
                nc.scalar.copy(vseg[:tl, i, 0:72], ps[:tl, 288:360])
                nc.scalar.copy(vseg[:tl, i, 97:169], ps[:tl, 360:432])

            pending = None
            for (t0, tl) in tiles:
                ps = qkv_mm(t0, tl)
                if pending is not None:
                    rope_tp(*pending)
                pending = (t0, tl, ps)
            if pending is not None:
                rope_tp(*pending)

        # ---------------- phase 2: attention + out-proj ------------------
        with ExitStack() as p2:
            P2 = lambda **kw: p2.enter_context(tc.tile_pool(**kw))
            ps_sc = P2(name="ps_sc", bufs=2, space="PSUM")   # 2 banks each
            ps_cx = P2(name="ps_cx", bufs=2, space="PSUM")
            ps_ou = P2(name="ps_ou", bufs=2, space="PSUM")
            esp = P2(name="esp", bufs=4)
            bcp = P2(name="bcp", bufs=3)
            crp = P2(name="crp", bufs=3)
            cxp = P2(name="cxp", bufs=8)
            osb = P2(name="osb", bufs=3)

            def attn_pair(a, b, pair, h):
                """scores+exp+PV for chunk pair, one head. Returns cxs."""
                sts = _stiles(a, b)
                totw = sum(cn for (c0, cn) in pair)
                cxs_out = []
                cx = {}
                for (c0, cn) in pair:
                    cx[c0] = ps_cx.tile([97, 512], F32, tag="cx",
                                        name=f"cx_{c0}_{h}")

                def pv_do(st, first, last):
                    s0, sn, es = st
                    i = tidx[s0]
                    off = 0
                    for (c0, cn) in pair:
                        nc.tensor.matmul(cx[c0][:, :cn],
                                         vseg[:sn, i, h * 97:(h + 1) * 97],
                                         es[:sn, off:off + cn],
                                         start=first, stop=last)
                        off += cn

                prev = None
                for si, (s0, sn) in enumerate(sts):
                    sc = ps_sc.tile([128, 1024], F32, tag="sc",
                                    name=f"sc_{pair[0][0]}_{h}_{si}")
                    off = 0
                    for (c0, cn) in pair:
                        nc.tensor.matmul(sc[:sn, off:off + cn],
                                         QK[:, 2 + h, s0:s0 + sn],
                                         QK[:, h, c0:c0 + cn],
                                         start=True, stop=True)
                        off += cn
                    es = esp.tile([128, 1024], BF16, tag="es",
                                  name=f"es_{pair[0][0]}_{h}_{si}")
                    nc.scalar.activation(es[:sn, :totw], sc[:sn, :totw],
                                         mybir.ActivationFunctionType.Exp,
                                         bias=ebias[:sn], scale=SCALE)
                    if prev is not None:
                        pv_do(prev, first=(si == 1), last=False)
                    prev = (s0, sn, es)
                pv_do(prev, first=(len(sts) == 1), last=True)

                for (c0, cn) in pair:
                    # free the cx bank early: raw evac + rowsum copy
                    # (GpSimd cannot read PSUM; rowsum copy goes to ACT)
                    rs = bcp.tile([1, 512], F32, tag="rs", name=f"rs_{c0}_{h}")
                    nc.scalar.copy(rs[:, :cn], cx[c0][96:97, :cn])
                    craw = crp.tile([HD, 512], BF16, tag="craw",
                                    name=f"craw_{c0}_{h}")
                    nc.vector.tensor_copy(craw[:, :cn], cx[c0][0:HD, :cn])
                    rr = bcp.tile([1, 512], F32, tag="rr", name=f"rr_{c0}_{h}")
                    nc.vector.reciprocal_approx_fast(out=rr[:, :cn],
                                                     in_=rs[:, :cn])
                    bc = bcp.tile([HD, 512], F32, tag="bc", name=f"bc_{c0}_{h}")
                    nc.gpsimd.partition_broadcast(bc[:, :cn], rr[:, :cn])
                    cxs = cxp.tile([HD, 512], BF16, tag="cxs",
                                   name=f"cxs_{c0}_{h}")
                    nc.vector.tensor_tensor(out=cxs[:, :cn], in0=craw[:, :cn],
                                            in1=bc[:, :cn],
                                            op=mybir.AluOpType.mult)
                    cxs_out.append(cxs)
                return cxs_out

            def outproj(pair, cxs01):
                for ci, (c0, cn) in enumerate(pair):
                    for m in range(9):
                        po = ps_ou.tile([128, 512], F32, tag="po",
                                        name=f"po_{c0}_{m}")
                        for h in range(HPC):
                            nc.tensor.matmul(po[:, :cn],
                                             wo_sb[:, h, m * 128:(m + 1) * 128],
                                             cxs01[h][ci][:, :cn],
                                             start=(h == 0), stop=(h == HPC - 1))
                        ob = osb.tile([128, 512], F32, tag="ob",
                                      name=f"ob_{c0}_{m}")
                        nc.vector.tensor_copy(ob[:, :cn], po[:, :cn])
                        nc.sync.dma_start(
                            out=outT[m * 128:(m + 1) * 128, c0:c0 + cn],
                            in_=ob[:, :cn])

            pend_out = None
            for a, b, chunks in plan:
                cl = [(c0, cn) for (c0, cn, tls) in chunks]
                pairs = [tuple(cl[i:i + 2]) for i in range(0, len(cl), 2)]
                for pair in pairs:
                    cxs01 = []
                    for h in range(HPC):
                        cxs01.append(attn_pair(a, b, pair, h))
                    if pend_out is not None:
                        outproj(*pend_out)
                    pend_out = (pair, cxs01)
            if pend_out is not None:
                outproj(*pend_out)
    return nc


def _build_inputs(x, wqkv, wout, cos, sin, plan):
    tiles = _all_tiles(plan)
    nt = len(tiles)
    bf = ml_dtypes.bfloat16

    # x packed per-partition-contiguous: xP[p, kt, t] = x[t, kt*128+p]
    xP = np.ascontiguousarray(
        x.reshape(T, 9, 128).transpose(2, 1, 0)).astype(bf)

    c = cos[:, :HD2]
    s = sin[:, :HD2]
    csP = np.zeros((128, nt, 2 * HD2), np.float32)
    for i, (t0, tl) in enumerate(tiles):
        csP[:tl, i, 0:HD2] = c[t0:t0 + tl]
        csP[:tl, i, HD2:2 * HD2] = s[t0:t0 + tl]
    csP = csP.astype(bf)
    idd = np.eye(128, dtype=np.float32).astype(bf)

    in_maps = []
    for core in range(NCORES):
        h0 = core * HPC
        rows = []
        for kind in range(3):
            for h in range(HPC):
                base = kind * H + (h0 + h) * HD
                rows.extend(range(base, base + HD))
        wq = np.ascontiguousarray(wqkv[rows, :].T)                 # [H, 432]
        wqP = np.ascontiguousarray(
            wq.reshape(9, 128, OC).transpose(1, 0, 2)).astype(bf)  # [128,9,432]
        cols = np.arange(h0 * HD, (h0 + HPC) * HD)
        wo = np.ascontiguousarray(wout[:, cols].T)                 # [144, H]
        woP = np.ascontiguousarray(
            wo.reshape(HPC, HD, H).transpose(1, 0, 2)).astype(bf)  # [72,2,H]
        in_maps.append({"xP": xP, "wqP": wqP, "woP": woP,
                        "csP": csP, "idd": idd})
    return in_maps


def kernel(hidden_states, wqkv, bqkv, wout, bout, cos, sin, cu_seqlens,
           _trace=False):
    x = np.asarray(hidden_states, np.float32).reshape(T, H)
    plan = _plan(np.asarray(cu_seqlens).astype(np.int64))
    if plan not in _CACHE:
        nc = bacc.Bacc("TRN2", target_bir_lowering=False, debug=False)
        build(nc, plan)
        nc.compile()
        _CACHE[plan] = nc
    nc = _CACHE[plan]
    in_maps = _build_inputs(x, np.asarray(wqkv, np.float32),
                            np.asarray(wout, np.float32),
                            np.asarray(cos, np.float32),
                            np.asarray(sin, np.float32), plan)
    res = bass_utils.run_bass_kernel_spmd(nc, in_maps,
                                          core_ids=list(range(NCORES)),
                                          trace=_trace)
    out = np.zeros((H, T), np.float64)
    for core in range(NCORES):
        out += res.results[core]["outT"].astype(np.float64)
    out = out.T + np.asarray(bout, np.float64)[None, :]
    if _trace:
        kernel.last_exec_time_ns = res.exec_time_ns
        kernel.last_trace = res.instructions_and_trace
    return out.astype(np.float32).reshape(1, T, H)
